# revision 21
# baseline (speedup 1.0000x reference)
"""GAU (gated attention unit) Trainium2 kernel, 8-way SPMD over the sequence dim.

Problem (fp32):
    h    = LayerNorm(x) * gamma + beta            x: [4096, 1024]
    uv   = silu(h @ uv_w.T + uv_b)                uv: [4096, 4224] = [u | v | base]
    q, k = base * qk_w[0,1] + qk_b[0,1]           base: [4096, 128]
    g    = relu(q @ k.T / sqrt(128))^2            g: [4096, 4096]
    out  = (u * (g @ v)) @ o_w.T + o_b + x        out: [4096, 1024]

Sharding: rows (sequence) split 8 ways; each core computes its own 512-row
slice of everything.  k and v are AllGathered across the 8 cores in two
row-chunked fp8 collectives: gatherA = [k | v rows rt0] fires as soon as the
first quarter of v is done, gatherB = [v rows rt1..rt3] follows.  The u
projection and the chunk-A attention accumulation fill the gather shadow.
Attention key-tile PAIRS are chosen so chunk-A pairs only touch rt0 rows
(kt, kt+16 stride pairing) and chunk-B pairs only touch rt1..rt3.

All projection matmuls run in fp8 DoubleRow (fp32 PSUM accumulation);
LayerNorm statistics, epilogues, and the residual stay fp32/bf16.  The
output is dominated by the fp32 residual chain, so fp8 rounding of the tiny
attention contribution is far below tolerance.

Scaling (powers of two, folded host-side where possible):
  weights *2^6 in fp8, undone by ACT scale 2^-6 at the silu
  q *2^7/sqrt(128), k *2^7  ->  scores pg = qk_true*2^14
  g = relu(pg)^2 = g_true*2^28 (fp8 absmax ~190)
  y = (attn*2^-10)*u = y_true*2^18 (fp8)
  o matmul: *2^6 weights -> po = out_true*2^24, undone at the residual add.
"""
import sys

sys.path.insert(0, "/opt/trn_rl_repo")

import numpy as np
import concourse.bass as bass
import concourse.tile as tile
from concourse import bacc, mybir
from concourse.bass_utils import run_bass_kernel_spmd

F32 = mybir.dt.float32
BF16 = mybir.dt.bfloat16
F8 = mybir.dt.float8e4
DR = mybir.MatmulPerfMode.DoubleRow
AF = mybir.ActivationFunctionType
OP = mybir.AluOpType

N_CORES = 8
N = 4096          # sequence
H = 1024          # hidden
E = 2048          # expansion
S = 128           # qk dim
R = N // N_CORES  # 512 rows per core
P = 128
EPS = 1e-5

HT = H // P       # 8  h-tiles
A = HT // 2       # 4  h pair-steps (DoubleRow)
RT = R // P       # 4  row tiles per core
UT = E // P       # 16 u col tiles
KT = N // P       # 32 key tiles
CBK = 32          # 32 rows per core in gatherK (fp8 k, 4 p-rows per row)
CBA = 2 * P       # 256 rows per core in gatherA (v rt0+rt1)
CBB = 2 * P       # 256 rows per core in gatherB (v rt2+rt3)


def build(vb_zero=False):
    nc = bacc.Bacc("TRN2", target_bir_lowering=False, debug=False,
                   num_devices=N_CORES)

    # ---- kernel I/O (per core) ----
    xt8_d = nc.declare_dram_parameter("xt8", [P, HT, R], F8, isOutput=False)
    xpb_d = nc.declare_dram_parameter("xpb", [R, H], F32, isOutput=False)
    wb_d = nc.declare_dram_parameter("wb8", [P, HT, S], F8, isOutput=False)
    wuv_d = nc.declare_dram_parameter("wuv8", [2, P, HT, E], F8,
                                      isOutput=False)
    wo_d = nc.declare_dram_parameter("wo8", [P, 2 * UT, H // 2], F8,
                                     isOutput=False)
    uvb_d = nc.declare_dram_parameter("uvb17", [P, UT + 1], F32,
                                      isOutput=False)
    cwb_d = nc.declare_dram_parameter("cwb", [S], F32, isOutput=False)
    vb_d = nc.declare_dram_parameter("vb", [E], F32, isOutput=False)
    qs_d = nc.declare_dram_parameter("qs", [S], F32, isOutput=False)
    qb_d = nc.declare_dram_parameter("qb", [S], F32, isOutput=False)
    ks_d = nc.declare_dram_parameter("ks", [S], F32, isOutput=False)
    kb_d = nc.declare_dram_parameter("kb", [S], F32, isOutput=False)
    out = nc.declare_dram_parameter("out", [R, H], F32, isOutput=True)

    outr = out.ap()

    from contextlib import ExitStack
    with tile.TileContext(nc) as tc, ExitStack() as ctx:
        singles = ctx.enter_context(tc.tile_pool(name="singles", bufs=1))
        tmp = ctx.enter_context(tc.tile_pool(name="tmp", bufs=2))
        vstr = ctx.enter_context(tc.tile_pool(name="vstr", bufs=6))
        ps = ctx.enter_context(tc.tile_pool(name="ps", bufs=2, space="PSUM"))
        dram = ctx.enter_context(tc.tile_pool(name="dram", bufs=1,
                                              space="DRAM"))

        # ---- big input DMAs first (sync queue = weight stream) ----
        xt_sb = singles.tile([P, HT, R], F8)
        nc.sync.dma_start(xt_sb, xt8_d.ap())
        wb_sb = singles.tile([P, HT, S], F8)
        nc.sync.dma_start(wb_sb, wb_d.ap())
        wv_sb = singles.tile([P, HT, E], F8)
        nc.sync.dma_start(wv_sb, wuv_d.ap()[0])
        wu_sb = singles.tile([P, HT, E], F8)
        nc.sync.dma_start(wu_sb, wuv_d.ap()[1])
        wo_sb = singles.tile([P, 2 * UT, H // 2], F8)
        nc.sync.dma_start(wo_sb, wo_d.ap())

        # ---- small constants on the scalar queue ----
        eps_t = singles.tile([P, 1], F32)
        nc.vector.memset(eps_t, EPS)
        dummy = singles.tile([P, 1], F32)
        # preload the ACT tables off the critical path
        nc.scalar.activation(out=dummy, in_=eps_t, func=AF.Square)
        nc.scalar.activation(out=dummy, in_=eps_t, func=AF.Sqrt)
        nc.scalar.activation(out=dummy, in_=eps_t, func=AF.Silu)
        uvb_sb = singles.tile([P, UT + 1], F32)
        nc.scalar.dma_start(uvb_sb, uvb_d.ap())
        qs_t = singles.tile([P, 1], F32)
        nc.scalar.dma_start(qs_t, qs_d.ap().rearrange("(t p) -> p t", p=P))
        qb_t = singles.tile([P, 1], F32)
        nc.scalar.dma_start(qb_t, qb_d.ap().rearrange("(t p) -> p t", p=P))
        ks_t = singles.tile([P, 1], F32)
        nc.scalar.dma_start(ks_t, ks_d.ap().rearrange("(t p) -> p t", p=P))
        kb_t = singles.tile([P, 1], F32)
        nc.scalar.dma_start(kb_t, kb_d.ap().rearrange("(t p) -> p t", p=P))
        cwb_t = singles.tile([P, 1], F32)
        nc.scalar.dma_start(cwb_t, cwb_d.ap().rearrange("(t p) -> p t", p=P))
        if not vb_zero:
            vb_bc = singles.tile([P, E], F32)
            nc.scalar.dma_start(vb_bc, vb_d.ap().partition_broadcast(P))
        xpb_sb = singles.tile([P, RT, H], F32)
        nc.scalar.dma_start(
            xpb_sb, xpb_d.ap().rearrange("(t p) c -> p t c", p=P))

        ones_s = singles.tile([P, 2, P], F8)
        nc.vector.memset(ones_s, 1.0)
        warm_mv = singles.tile([P, 2, R], F8)
        nc.vector.memset(warm_mv, 0.0)

        # ---- persistent activations ----
        hT = singles.tile([P, HT, R], F8)              # normalized x (f8)
        baseT = singles.tile([P, R], F32)
        qT = singles.tile([P, R], F8)
        kT_sb = singles.tile([P, R], F8)
        v_sb = singles.tile([P, RT, E], F8)            # v, natural layout
        uT = singles.tile([P, UT, R], F8)              # u, later y=u*attn
        g_sb = singles.tile([P, KT, R], F8)            # relu(qk)^2 scaled
        kT_full = singles.tile([P, N_CORES, R], F8)    # gathered k

        # ---- internal DRAM for the three row-chunked AllGathers ----
        contribK = dram.tile([CBK, E], F8)
        gatherK = dram.tile([N_CORES * CBK, E], F8, addr_space="Shared")
        contribA = dram.tile([CBA, E], F8)
        gatherA = dram.tile([N_CORES * CBA, E], F8, addr_space="Shared")
        contribB = dram.tile([CBB, E], F8)
        gatherB = dram.tile([N_CORES * CBB, E], F8, addr_space="Shared")

        # ---- PE warm-up: ~5us of junk matmuls while xt streams in ----
        warm_ps = ps.tile([P, 4, R], F32, tag="mm4", name="warm")
        for i in range(12):
            nc.tensor.matmul(warm_ps[:, 0, :], ones_s, warm_mv,
                             perf_mode=DR, start=(i == 0), stop=(i == 11))

        # ================= Stage 1: LayerNorm stats (transposed) ==========
        # x arrives host-transposed and fp8-quantized as xt [128, ht, 512]
        # with h = ht*128 + p.  Sums over h via all-ones DoubleRow matmuls
        # land pre-broadcast on 128 partitions.
        xsq = singles.tile([P, HT, R], F8)
        for a in range(A):
            # (x*0.25)*x = x^2/4 (fp8-safe range)
            nc.vector.scalar_tensor_tensor(
                out=xsq[:, 2 * a:2 * a + 2, :],
                in0=xt_sb[:, 2 * a:2 * a + 2, :], scalar=0.25,
                in1=xt_sb[:, 2 * a:2 * a + 2, :], op0=OP.mult, op1=OP.mult)
        sq2 = ps.tile([P, 4, R], F32, tag="mm4", name="sq2")
        psum_s = sq2[:, 0, :]
        psum_q = sq2[:, 1, :]
        for a in range(A):
            nc.tensor.matmul(psum_s, ones_s, xt_sb[:, 2 * a:2 * a + 2, :],
                             perf_mode=DR, start=(a == 0), stop=(a == A - 1))
        # base projection straight off raw fp8 x; normalization is fixed
        # up on the single output tile (pre = rstd*(x@Wb) + nmr*colsum_Wb)
        pb4 = ps.tile([P, 4, R], F32, tag="mm4", name="pb4")
        pb = pb4[:, 0, :]
        for a in range(A):
            nc.tensor.matmul(pb, wb_sb[:, 2 * a:2 * a + 2, :],
                             xt_sb[:, 2 * a:2 * a + 2, :],
                             perf_mode=DR, start=(a == 0), stop=(a == A - 1))
        for a in range(A):
            nc.tensor.matmul(psum_q, ones_s, xsq[:, 2 * a:2 * a + 2, :],
                             perf_mode=DR, start=(a == 0), stop=(a == A - 1))
        # keep the PE (and HAM) busy while the LayerNorm chain runs
        junk_ps = ps.tile([P, 4, R], F32, tag="mm4", name="junk")
        for i in range(48):
            nc.tensor.matmul(junk_ps[:, 0, :P], ones_s, warm_mv[:, :, :P],
                             perf_mode=DR, start=(i == 0), stop=(i == 47))
        # mu = sum_x/H; var = sum_q/256 - mu^2; rstd = 1/sqrt(var+eps)
        mu2 = tmp.tile([P, R], F32, tag="stat", bufs=1, name="mu2")
        nc.scalar.activation(out=mu2, in_=psum_s, func=AF.Square,
                             scale=1.0 / H)
        var_t = tmp.tile([P, R], F32, tag="stat2", bufs=1, name="var")
        nc.vector.scalar_tensor_tensor(
            out=var_t, in0=psum_q, scalar=1.0 / 256.0, in1=mu2,
            op0=OP.mult, op1=OP.subtract)
        rstd = tmp.tile([P, R], F32, tag="stat", bufs=1, name="rstd")
        nc.scalar.activation(out=rstd, in_=var_t, func=AF.Sqrt,
                             bias=eps_t, scale=1.0)
        nc.vector.reciprocal_approx_fast(out=rstd, in_=rstd)
        nmr = tmp.tile([P, R], BF16, tag="stat2", bufs=1, name="nmr")
        nc.vector.scalar_tensor_tensor(
            out=nmr, in0=psum_s, scalar=-1.0 / H, in1=rstd,
            op0=OP.mult, op1=OP.mult)
        # ================= Stage 2a: base fixup -> q,k -> k gather ========
        t1 = tmp.tile([P, R], F32, tag="bfix", bufs=1, name="t1")
        nc.vector.tensor_scalar_mul(t1, nmr, cwb_t)
        t2 = tmp.tile([P, R], F32, tag="bfix2", bufs=1, name="t2")
        nc.vector.scalar_tensor_tensor(
            out=t2, in0=pb, scalar=2.0 ** -6, in1=rstd,
            op0=OP.mult, op1=OP.mult)
        pre_b = tmp.tile([P, R], F32, tag="bfix3", bufs=1, name="pre_b")
        nc.vector.tensor_tensor(pre_b, t2, t1, OP.add)
        nc.scalar.activation(out=baseT, in_=pre_b, func=AF.Silu,
                             bias=uvb_sb[:, UT:UT + 1], scale=1.0)

        # hT = xt*rstd + nmr, per h-pair so the projections start early;
        # q/k land right after the first pair so the k gather fires early
        rstd_b2 = rstd[:].unsqueeze(1).broadcast_to([P, 2, R])
        nmr_b2 = nmr[:].unsqueeze(1).broadcast_to([P, 2, R])

        def norm_pair(a):
            d = tmp.tile([P, 2, R], BF16, tag="norm", bufs=2)
            nc.vector.tensor_tensor(d, xt_sb[:, 2 * a:2 * a + 2, :],
                                    rstd_b2, OP.mult)
            nc.vector.tensor_tensor(hT[:, 2 * a:2 * a + 2, :], d,
                                    nmr_b2, OP.add)

        norm_pair(0)
        nc.vector.tensor_scalar(out=qT, in0=baseT, scalar1=qs_t, scalar2=qb_t,
                                op0=OP.mult, op1=OP.add)
        nc.vector.tensor_scalar(out=kT_sb, in0=baseT, scalar1=ks_t,
                                scalar2=kb_t, op0=OP.mult, op1=OP.add)
        # f8 k [128,512] packed into 32 rows of 2048 (sync queue: hw DGE)
        nc.sync.dma_start(
            contribK[:].rearrange("r (four c) -> (r four) c", four=4),
            kT_sb[:])
        nc.gpsimd.collective_compute(
            "AllGather", OP.bypass,
            replica_groups=[list(range(N_CORES))],
            ins=[contribK.opt()], outs=[gatherK.opt()])
        for a in range(1, A):
            norm_pair(a)

        # ================= Stage 2b: v, two row-chunked gathers ===========
        def v_rows(rt):
            pv = ps.tile([P, 4, 512], F32, tag="mm4", name=f"pv{rt}")
            for a in range(A):
                for ci in range(4):
                    nc.tensor.matmul(
                        pv[:, ci, :],
                        hT[:, 2 * a:2 * a + 2, rt * P:(rt + 1) * P],
                        wv_sb[:, 2 * a:2 * a + 2, ci * 512:(ci + 1) * 512],
                        perf_mode=DR, start=(a == 0), stop=(a == A - 1))
            pv_w = pv[:].rearrange("p a b -> p (a b)")
            if vb_zero:
                for hf in range(2):
                    nc.scalar.activation(
                        out=v_sb[:, rt, hf * 1024:(hf + 1) * 1024],
                        in_=pv_w[:, hf * 1024:(hf + 1) * 1024],
                        func=AF.Silu, scale=2.0 ** -6)
            else:
                tv = tmp.tile([P, E], F32, tag="vtmp", bufs=2)
                nc.vector.scalar_tensor_tensor(
                    out=tv, in0=pv_w, scalar=2.0 ** -6, in1=vb_bc,
                    op0=OP.mult, op1=OP.add)
                nc.scalar.activation(out=v_sb[:, rt, :], in_=tv,
                                     func=AF.Silu)

        for rt in range(2):
            v_rows(rt)
            nc.sync.dma_start(contribA[rt * P:(rt + 1) * P, :],
                              v_sb[:, rt, :])
        nc.gpsimd.collective_compute(
            "AllGather", OP.bypass,
            replica_groups=[list(range(N_CORES))],
            ins=[contribA.opt()], outs=[gatherA.opt()])
        for rt in range(2, RT):
            v_rows(rt)
            # sync queue: can't be head-blocked by collective A
            nc.sync.dma_start(
                contribB[(rt - 2) * P:(rt - 1) * P, :], v_sb[:, rt, :])
        nc.gpsimd.collective_compute(
            "AllGather", OP.bypass,
            replica_groups=[list(range(N_CORES))],
            ins=[contribB.opt()], outs=[gatherB.opt()])

        # ================= Stage 2c: u (fills the gather shadow) ==========
        for ci in range(4):
            pu4 = ps.tile([P, 4, R], F32, tag="mm4", name=f"pu{ci}")
            for ui in range(4):
                ut = ci * 4 + ui
                for a in range(A):
                    nc.tensor.matmul(
                        pu4[:, ui, :],
                        wu_sb[:, 2 * a:2 * a + 2, ut * P:(ut + 1) * P],
                        hT[:, 2 * a:2 * a + 2, :],
                        perf_mode=DR, start=(a == 0), stop=(a == A - 1))
            for ui in range(4):
                ut = ci * 4 + ui
                nc.scalar.activation(out=uT[:, ut, :], in_=pu4[:, ui, :],
                                     func=AF.Silu,
                                     bias=uvb_sb[:, ut:ut + 1],
                                     scale=2.0 ** -6)

        # ================= Stage 3: scores + relu^2 =======================
        # gatherK block for core c: 32 rows of f8 k (4 p-rows per row)
        nc.sync.dma_start(
            kT_full,
            gatherK[:].rearrange("(c b) (four w) -> (b four) c w",
                                 b=CBK, four=4)[:P])
        for kq in range(KT // 4):
            pg = ps.tile([P, 4, R], F32, tag="mm4", name=f"pg{kq}")
            for j in range(4):
                kt = 4 * kq + j
                c, rb = kt // RT, kt % RT
                nc.tensor.matmul(pg[:, j, :],
                                 kT_full[:, c, rb * P:(rb + 1) * P],
                                 qT[:], start=True, stop=True)
            t_relu = tmp.tile([P, 4, R], BF16, tag="relu", bufs=2)
            nc.vector.tensor_scalar_max(t_relu, pg, 0.0)
            nc.vector.tensor_tensor(g_sb[:, 4 * kq:4 * kq + 4, :],
                                    t_relu, t_relu, OP.mult)

        # ================= Stage 4: attn = g @ v; y = u * attn ===========
        # fp8 DoubleRow: each matmul contracts a PAIR of adjacent key
        # tiles (256 keys).  A-pairs (4c, 4c+1) live in gatherA block c;
        # B-pairs (4c+2, 4c+3) live in gatherB block c.
        def stripe(gsrc, c, ch):
            st = vstr.tile([P, 2, 1024], F8, tag="vstripe")
            nc.gpsimd.dma_start(
                st, gsrc[c * 2 * P:(c + 1) * 2 * P,
                         ch * 1024:(ch + 1) * 1024]
                .rearrange("(a p) e -> p a e", a=2))
            return st

        EC = 8
        for ch in range(2):
            pa_lo = ps.tile([P, 4, R], F32, tag="mm4", name=f"pa{ch}lo")
            pa_hi = ps.tile([P, 4, R], F32, tag="mm4", name=f"pa{ch}hi")
            steps = ([(gatherA, c, g_sb[:, 4 * c:4 * c + 2, :])
                      for c in range(N_CORES)]
                     + [(gatherB, c, g_sb[:, 4 * c + 2:4 * c + 4, :])
                        for c in range(N_CORES)])
            n_steps = len(steps)
            for si, (gsrc, idx, gpair) in enumerate(steps):
                st = stripe(gsrc, idx, ch)
                for ei in range(EC):
                    pa = pa_lo if ei < 4 else pa_hi
                    nc.tensor.matmul(pa[:, ei % 4, :],
                                     st[:, :, ei * P:(ei + 1) * P],
                                     gpair,
                                     perf_mode=DR,
                                     start=(si == 0),
                                     stop=(si == n_steps - 1))
            for half, pa in enumerate((pa_lo, pa_hi)):
                usl = slice(ch * EC + half * 4, ch * EC + half * 4 + 4)
                nc.vector.scalar_tensor_tensor(
                    out=uT[:, usl, :], in0=pa, scalar=2.0 ** -10,
                    in1=uT[:, usl, :], op0=OP.mult, op1=OP.mult)

        # ================= Stage 5: out = y @ o_w.T + o_b + x ============
        outr3 = outr[:].rearrange("(t p) c -> p t c", p=P)
        for hc in range(2):
            po4 = ps.tile([P, 4, 512], F32, tag="mm4", name=f"po{hc}")
            for rt in range(RT):
                for t in range(UT // 2):
                    nc.tensor.matmul(
                        po4[:, rt, :],
                        uT[:, 2 * t:2 * t + 2, rt * P:(rt + 1) * P],
                        wo_sb[:, hc * UT + 2 * t:hc * UT + 2 * t + 2, :],
                        perf_mode=DR, start=(t == 0), stop=(t == UT // 2 - 1))
            for hh in range(2):
                o_sb = tmp.tile([P, 2, 512], F32, tag="osb")
                nc.vector.scalar_tensor_tensor(
                    out=o_sb, in0=po4[:, 2 * hh:2 * hh + 2, :],
                    scalar=2.0 ** -24,
                    in1=xpb_sb[:, 2 * hh:2 * hh + 2,
                               hc * 512:(hc + 1) * 512],
                    op0=OP.mult, op1=OP.add)
                nc.sync.dma_start(
                    outr3[:, 2 * hh:2 * hh + 2, hc * 512:(hc + 1) * 512],
                    o_sb)

    nc.finalize()
    return nc


_NC_CACHE = {}


def _get_nc(vb_zero):
    if vb_zero not in _NC_CACHE:
        _NC_CACHE[vb_zero] = build(vb_zero)
    return _NC_CACHE[vb_zero]


def _make_in_maps(inputs):
    import ml_dtypes
    f8 = ml_dtypes.float8_e4m3fn
    x = np.asarray(inputs["x"], dtype=np.float32)
    uv_w = np.asarray(inputs["uv_w"], dtype=np.float32)
    o_w = np.asarray(inputs["o_w"], dtype=np.float32)
    qk_w = np.asarray(inputs["qk_weight"], dtype=np.float32)
    qk_b = np.asarray(inputs["qk_bias"], dtype=np.float32)
    gamma = np.asarray(inputs["ln_gamma"], dtype=np.float32)
    beta = np.asarray(inputs["ln_beta"], dtype=np.float32)
    uv_b = np.asarray(inputs["uv_b"], dtype=np.float32)
    o_b = np.asarray(inputs["o_b"], dtype=np.float32)
    sq = np.float32(1.0 / np.sqrt(np.float32(128.0)))

    # fold gamma/beta into the uv projection:
    #   (z*gamma + beta) @ W.T = z @ (W*gamma).T + W@beta
    uv_w_f = uv_w * gamma[None, :]
    uv_b_f = (uv_b.astype(np.float64)
              + uv_w.astype(np.float64) @ beta.astype(np.float64)
              ).astype(np.float32)

    def to_pht(w, cols):
        # [cols, H] weight rows -> [P, HT, cols] with h = ht*128 + p
        return np.ascontiguousarray(
            w.T.reshape(HT, P, cols).transpose(1, 0, 2))

    wb8 = (to_pht(uv_w_f[2 * E:], S) * 64.0).astype(f8)
    wuv8 = np.stack([
        (to_pht(uv_w_f[E:2 * E], E) * 64.0).astype(f8),
        (to_pht(uv_w_f[:E], E) * 64.0).astype(f8)])
    # o_w [H, E] -> [P, 2*UT, 512] with (hc, et) interleaved: index
    # hc*UT + et, e = et*128 + p, columns = hc*512 + c
    wo = o_w.T.reshape(UT, P, 2, 512).transpose(1, 2, 0, 3).reshape(
        P, 2 * UT, 512)
    wo8 = np.ascontiguousarray(wo * 64.0).astype(f8)
    uvb17 = np.concatenate(
        [uv_b_f[:E].reshape(UT, P).T, uv_b_f[2 * E:].reshape(1, P).T],
        axis=1).astype(np.float32)
    uvb17 = np.ascontiguousarray(uvb17)

    shared = dict(
        wb8=wb8, wuv8=wuv8, wo8=wo8, uvb17=uvb17,
        vb=np.ascontiguousarray(uv_b_f[E:2 * E]),
        qs=np.ascontiguousarray(qk_w[0] * sq * 128.0),
        qb=np.ascontiguousarray(qk_b[0] * sq * 128.0),
        ks=np.ascontiguousarray(qk_w[1] * 128.0),
        kb=np.ascontiguousarray(qk_b[1] * 128.0),
        cwb=np.ascontiguousarray(uv_w_f[2 * E:].sum(axis=1)
                                 .astype(np.float32)),
    )
    xpb = x + o_b[None, :]
    maps = []
    for c in range(N_CORES):
        xc = x[c * R:(c + 1) * R]
        xt8 = np.ascontiguousarray(
            xc.T.reshape(HT, P, R).transpose(1, 0, 2)).astype(f8)
        maps.append(dict(shared, xt8=xt8,
                         xpb=np.ascontiguousarray(xpb[c * R:(c + 1) * R])))
    return maps


def run(inputs, trace=False, **kw):
    in_maps = _make_in_maps(inputs)
    nc = _get_nc(bool(np.all(in_maps[0]["vb"] == 0.0)))
    res = run_bass_kernel_spmd(nc, in_maps, list(range(N_CORES)),
                               trace=trace, **kw)
    out = np.concatenate([res.results[c]["out"] for c in range(N_CORES)],
                         axis=0)
    return out, res


def kernel(**inputs) -> np.ndarray:
    out, _ = run(inputs)
    return out


# revision 22
# speedup vs baseline: 1.1207x; 1.1207x over previous
"""GAU (gated attention unit) Trainium2 kernel, 8-way SPMD over the sequence dim.

Problem (fp32):
    h    = LayerNorm(x) * gamma + beta            x: [4096, 1024]
    uv   = silu(h @ uv_w.T + uv_b)                uv: [4096, 4224] = [u | v | base]
    q, k = base * qk_w[0,1] + qk_b[0,1]           base: [4096, 128]
    g    = relu(q @ k.T / sqrt(128))^2            g: [4096, 4096]
    out  = (u * (g @ v)) @ o_w.T + o_b + x        out: [4096, 1024]

Sharding: rows (sequence) split 8 ways; each core computes its own 512-row
slice of everything.  k and v are AllGathered across the 8 cores in two
row-chunked fp8 collectives: gatherA = [k | v rows rt0] fires as soon as the
first quarter of v is done, gatherB = [v rows rt1..rt3] follows.  The u
projection and the chunk-A attention accumulation fill the gather shadow.
Attention key-tile PAIRS are chosen so chunk-A pairs only touch rt0 rows
(kt, kt+16 stride pairing) and chunk-B pairs only touch rt1..rt3.

All projection matmuls run in fp8 DoubleRow (fp32 PSUM accumulation);
LayerNorm statistics, epilogues, and the residual stay fp32/bf16.  The
output is dominated by the fp32 residual chain, so fp8 rounding of the tiny
attention contribution is far below tolerance.

Scaling (powers of two, folded host-side where possible):
  weights *2^6 in fp8, undone by ACT scale 2^-6 at the silu
  q *2^7/sqrt(128), k *2^7  ->  scores pg = qk_true*2^14
  g = relu(pg)^2 = g_true*2^28 (fp8 absmax ~190)
  y = (attn*2^-10)*u = y_true*2^18 (fp8)
  o matmul: *2^6 weights -> po = out_true*2^24, undone at the residual add.
"""
import sys

sys.path.insert(0, "/opt/trn_rl_repo")

import numpy as np
import concourse.bass as bass
import concourse.tile as tile
from concourse import bacc, mybir
from concourse.bass_utils import run_bass_kernel_spmd

F32 = mybir.dt.float32
BF16 = mybir.dt.bfloat16
F8 = mybir.dt.float8e4
DR = mybir.MatmulPerfMode.DoubleRow
AF = mybir.ActivationFunctionType
OP = mybir.AluOpType

N_CORES = 8
N = 4096          # sequence
H = 1024          # hidden
E = 2048          # expansion
S = 128           # qk dim
R = N // N_CORES  # 512 rows per core
P = 128
EPS = 1e-5

HT = H // P       # 8  h-tiles
A = HT // 2       # 4  h pair-steps (DoubleRow)
RT = R // P       # 4  row tiles per core
UT = E // P       # 16 u col tiles
KT = N // P       # 32 key tiles
CBK = 32          # 32 rows per core in gatherK (fp8 k, 4 p-rows per row)
CBA = 2 * P       # 256 rows per core in gatherA (v rt0+rt1)
CBB = 2 * P       # 256 rows per core in gatherB (v rt2+rt3)


def build(vb_zero=False):
    nc = bacc.Bacc("TRN2", target_bir_lowering=False, debug=False,
                   num_devices=N_CORES)

    # ---- kernel I/O (per core) ----
    xt8_d = nc.declare_dram_parameter("xt8", [P, HT, R], F8, isOutput=False)
    xpb_d = nc.declare_dram_parameter("xpb", [R, H], F32, isOutput=False)
    wb_d = nc.declare_dram_parameter("wb8", [P, HT, S], F8, isOutput=False)
    wuv_d = nc.declare_dram_parameter("wuv8", [2, P, HT, E], F8,
                                      isOutput=False)
    wo_d = nc.declare_dram_parameter("wo8", [P, 2 * UT, H // 2], F8,
                                     isOutput=False)
    uvb_d = nc.declare_dram_parameter("uvb17", [P, UT + 1], F32,
                                      isOutput=False)
    cwb_d = nc.declare_dram_parameter("cwb", [S], F32, isOutput=False)
    vb_d = nc.declare_dram_parameter("vb", [E], F32, isOutput=False)
    qs_d = nc.declare_dram_parameter("qs", [S], F32, isOutput=False)
    qb_d = nc.declare_dram_parameter("qb", [S], F32, isOutput=False)
    ks_d = nc.declare_dram_parameter("ks", [S], F32, isOutput=False)
    kb_d = nc.declare_dram_parameter("kb", [S], F32, isOutput=False)
    out = nc.declare_dram_parameter("out", [R, H], F32, isOutput=True)

    outr = out.ap()

    from contextlib import ExitStack
    with tile.TileContext(nc) as tc, ExitStack() as ctx:
        singles = ctx.enter_context(tc.tile_pool(name="singles", bufs=1))
        tmp = ctx.enter_context(tc.tile_pool(name="tmp", bufs=2))
        vstr = ctx.enter_context(tc.tile_pool(name="vstr", bufs=6))
        ps = ctx.enter_context(tc.tile_pool(name="ps", bufs=2, space="PSUM"))
        dram = ctx.enter_context(tc.tile_pool(name="dram", bufs=1,
                                              space="DRAM"))

        # ---- big input DMAs first (sync queue = weight stream) ----
        xt_sb = singles.tile([P, HT, R], F8)
        nc.sync.dma_start(xt_sb, xt8_d.ap())
        wb_sb = singles.tile([P, HT, S], F8)
        nc.sync.dma_start(wb_sb, wb_d.ap())
        wv_sb = singles.tile([P, HT, E], F8)
        nc.sync.dma_start(wv_sb, wuv_d.ap()[0])
        wu_sb = singles.tile([P, HT, E], F8)
        nc.sync.dma_start(wu_sb, wuv_d.ap()[1])
        wo_sb = singles.tile([P, 2 * UT, H // 2], F8)
        nc.sync.dma_start(wo_sb, wo_d.ap())

        # ---- small constants on the scalar queue ----
        eps_t = singles.tile([P, 1], F32)
        nc.vector.memset(eps_t, EPS)
        dummy = singles.tile([P, 1], F32)
        # preload the ACT tables off the critical path
        nc.scalar.activation(out=dummy, in_=eps_t, func=AF.Square)
        nc.scalar.activation(out=dummy, in_=eps_t, func=AF.Sqrt)
        nc.scalar.activation(out=dummy, in_=eps_t, func=AF.Silu)
        uvb_sb = singles.tile([P, UT + 1], F32)
        nc.scalar.dma_start(uvb_sb, uvb_d.ap())
        qs_t = singles.tile([P, 1], F32)
        nc.scalar.dma_start(qs_t, qs_d.ap().rearrange("(t p) -> p t", p=P))
        qb_t = singles.tile([P, 1], F32)
        nc.scalar.dma_start(qb_t, qb_d.ap().rearrange("(t p) -> p t", p=P))
        ks_t = singles.tile([P, 1], F32)
        nc.scalar.dma_start(ks_t, ks_d.ap().rearrange("(t p) -> p t", p=P))
        kb_t = singles.tile([P, 1], F32)
        nc.scalar.dma_start(kb_t, kb_d.ap().rearrange("(t p) -> p t", p=P))
        cwb_t = singles.tile([P, 1], F32)
        nc.scalar.dma_start(cwb_t, cwb_d.ap().rearrange("(t p) -> p t", p=P))
        if not vb_zero:
            vb_bc = singles.tile([P, E], F32)
            nc.scalar.dma_start(vb_bc, vb_d.ap().partition_broadcast(P))
        xpb_sb = singles.tile([P, RT, H], F32)
        nc.scalar.dma_start(
            xpb_sb, xpb_d.ap().rearrange("(t p) c -> p t c", p=P))

        ones_s = singles.tile([P, 2, P], F8)
        nc.vector.memset(ones_s, 1.0)
        warm_mv = singles.tile([P, 2, R], F8)
        nc.vector.memset(warm_mv, 0.0)

        # ---- persistent activations ----
        hT = singles.tile([P, HT, R], F8)              # normalized x (f8)
        baseT = singles.tile([P, R], F32)
        qT = singles.tile([P, R], F8)
        kT_sb = singles.tile([P, R], F8)
        v_sb = singles.tile([P, RT, E], F8)            # v, natural layout
        uT = singles.tile([P, UT, R], F8)              # u
        y_sb = singles.tile([P, UT, R], F8)            # y = u*attn, accum
        g_sb = singles.tile([P, KT, R], F8)            # relu(qk)^2 scaled
        kT_full = singles.tile([P, N_CORES, R], F8)    # gathered k

        # ---- internal DRAM for the three row-chunked AllGathers ----
        contribK = dram.tile([CBK, E], F8)
        gatherK = dram.tile([N_CORES * CBK, E], F8, addr_space="Shared")
        contribA = dram.tile([CBA, E], F8)
        gatherA = dram.tile([N_CORES * CBA, E], F8, addr_space="Shared")
        contribB = dram.tile([CBB, E], F8)
        gatherB = dram.tile([N_CORES * CBB, E], F8, addr_space="Shared")

        # ---- PE warm-up: ~5us of junk matmuls while xt streams in ----
        warm_ps = ps.tile([P, 4, R], F32, tag="mm4", name="warm")
        for i in range(12):
            nc.tensor.matmul(warm_ps[:, 0, :], ones_s, warm_mv,
                             perf_mode=DR, start=(i == 0), stop=(i == 11))

        # ================= Stage 1: LayerNorm stats (transposed) ==========
        # x arrives host-transposed and fp8-quantized as xt [128, ht, 512]
        # with h = ht*128 + p.  Sums over h via all-ones DoubleRow matmuls
        # land pre-broadcast on 128 partitions.
        xsq = singles.tile([P, HT, R], F8)
        for a in range(A):
            # (x*0.25)*x = x^2/4 (fp8-safe range)
            nc.vector.scalar_tensor_tensor(
                out=xsq[:, 2 * a:2 * a + 2, :],
                in0=xt_sb[:, 2 * a:2 * a + 2, :], scalar=0.25,
                in1=xt_sb[:, 2 * a:2 * a + 2, :], op0=OP.mult, op1=OP.mult)
        sq2 = ps.tile([P, 4, R], F32, tag="mm4", name="sq2")
        psum_s = sq2[:, 0, :]
        psum_q = sq2[:, 1, :]
        for a in range(A):
            nc.tensor.matmul(psum_s, ones_s, xt_sb[:, 2 * a:2 * a + 2, :],
                             perf_mode=DR, start=(a == 0), stop=(a == A - 1))
        # base projection straight off raw fp8 x; normalization is fixed
        # up on the single output tile (pre = rstd*(x@Wb) + nmr*colsum_Wb)
        pb4 = ps.tile([P, 4, R], F32, tag="mm4", name="pb4")
        pb = pb4[:, 0, :]
        for a in range(A):
            nc.tensor.matmul(pb, wb_sb[:, 2 * a:2 * a + 2, :],
                             xt_sb[:, 2 * a:2 * a + 2, :],
                             perf_mode=DR, start=(a == 0), stop=(a == A - 1))
        for a in range(A):
            nc.tensor.matmul(psum_q, ones_s, xsq[:, 2 * a:2 * a + 2, :],
                             perf_mode=DR, start=(a == 0), stop=(a == A - 1))
        # keep the PE (and HAM) busy while the LayerNorm chain runs
        junk_ps = ps.tile([P, 4, R], F32, tag="mm4", name="junk")
        for i in range(48):
            nc.tensor.matmul(junk_ps[:, 0, :P], ones_s, warm_mv[:, :, :P],
                             perf_mode=DR, start=(i == 0), stop=(i == 47))
        # mu = sum_x/H; var = sum_q/256 - mu^2; rstd = 1/sqrt(var+eps)
        mu2 = tmp.tile([P, R], F32, tag="stat", bufs=1, name="mu2")
        nc.scalar.activation(out=mu2, in_=psum_s, func=AF.Square,
                             scale=1.0 / H)
        var_t = tmp.tile([P, R], F32, tag="stat2", bufs=1, name="var")
        nc.vector.scalar_tensor_tensor(
            out=var_t, in0=psum_q, scalar=1.0 / 256.0, in1=mu2,
            op0=OP.mult, op1=OP.subtract)
        rstd = tmp.tile([P, R], F32, tag="stat", bufs=1, name="rstd")
        nc.scalar.activation(out=rstd, in_=var_t, func=AF.Sqrt,
                             bias=eps_t, scale=1.0)
        nc.vector.reciprocal_approx_fast(out=rstd, in_=rstd)
        nmr = tmp.tile([P, R], BF16, tag="stat2", bufs=1, name="nmr")
        nc.vector.scalar_tensor_tensor(
            out=nmr, in0=psum_s, scalar=-1.0 / H, in1=rstd,
            op0=OP.mult, op1=OP.mult)
        # ================= Stage 2a: base fixup -> q,k -> k gather ========
        t1 = tmp.tile([P, R], F32, tag="bfix", bufs=1, name="t1")
        nc.vector.tensor_scalar_mul(t1, nmr, cwb_t)
        t2 = tmp.tile([P, R], F32, tag="bfix2", bufs=1, name="t2")
        nc.vector.scalar_tensor_tensor(
            out=t2, in0=pb, scalar=2.0 ** -6, in1=rstd,
            op0=OP.mult, op1=OP.mult)
        pre_b = tmp.tile([P, R], F32, tag="bfix3", bufs=1, name="pre_b")
        nc.vector.tensor_tensor(pre_b, t2, t1, OP.add)
        nc.scalar.activation(out=baseT, in_=pre_b, func=AF.Silu,
                             bias=uvb_sb[:, UT:UT + 1], scale=1.0)

        # hT = xt*rstd + nmr, per h-pair so the projections start early;
        # q/k land right after the first pair so the k gather fires early
        rstd_b2 = rstd[:].unsqueeze(1).broadcast_to([P, 2, R])
        nmr_b2 = nmr[:].unsqueeze(1).broadcast_to([P, 2, R])

        def norm_pair(a):
            d = tmp.tile([P, 2, R], BF16, tag="norm", bufs=2)
            nc.vector.tensor_tensor(d, xt_sb[:, 2 * a:2 * a + 2, :],
                                    rstd_b2, OP.mult)
            nc.vector.tensor_tensor(hT[:, 2 * a:2 * a + 2, :], d,
                                    nmr_b2, OP.add)

        norm_pair(0)
        nc.vector.tensor_scalar(out=qT, in0=baseT, scalar1=qs_t, scalar2=qb_t,
                                op0=OP.mult, op1=OP.add)
        nc.vector.tensor_scalar(out=kT_sb, in0=baseT, scalar1=ks_t,
                                scalar2=kb_t, op0=OP.mult, op1=OP.add)
        # f8 k [128,512] packed into 32 rows of 2048 (sync queue: hw DGE)
        nc.sync.dma_start(
            contribK[:].rearrange("r (four c) -> (r four) c", four=4),
            kT_sb[:])
        nc.gpsimd.collective_compute(
            "AllGather", OP.bypass,
            replica_groups=[list(range(N_CORES))],
            ins=[contribK.opt()], outs=[gatherK.opt()])
        for a in range(1, A):
            norm_pair(a)

        # ================= Stage 2b: v, two row-chunked gathers ===========
        def v_rows(rt):
            pv = ps.tile([P, 4, 512], F32, tag="mm4", name=f"pv{rt}")
            for a in range(A):
                for ci in range(4):
                    nc.tensor.matmul(
                        pv[:, ci, :],
                        hT[:, 2 * a:2 * a + 2, rt * P:(rt + 1) * P],
                        wv_sb[:, 2 * a:2 * a + 2, ci * 512:(ci + 1) * 512],
                        perf_mode=DR, start=(a == 0), stop=(a == A - 1))
            pv_w = pv[:].rearrange("p a b -> p (a b)")
            if vb_zero:
                for hf in range(2):
                    nc.scalar.activation(
                        out=v_sb[:, rt, hf * 1024:(hf + 1) * 1024],
                        in_=pv_w[:, hf * 1024:(hf + 1) * 1024],
                        func=AF.Silu, scale=2.0 ** -6)
            else:
                tv = tmp.tile([P, E], F32, tag="vtmp", bufs=2)
                nc.vector.scalar_tensor_tensor(
                    out=tv, in0=pv_w, scalar=2.0 ** -6, in1=vb_bc,
                    op0=OP.mult, op1=OP.add)
                nc.scalar.activation(out=v_sb[:, rt, :], in_=tv,
                                     func=AF.Silu)

        for rt in range(2):
            v_rows(rt)
            nc.sync.dma_start(contribA[rt * P:(rt + 1) * P, :],
                              v_sb[:, rt, :])
        nc.gpsimd.collective_compute(
            "AllGather", OP.bypass,
            replica_groups=[list(range(N_CORES))],
            ins=[contribA.opt()], outs=[gatherA.opt()])
        for rt in range(2, RT):
            v_rows(rt)
            # sync queue: can't be head-blocked by collective A
            nc.sync.dma_start(
                contribB[(rt - 2) * P:(rt - 1) * P, :], v_sb[:, rt, :])
        nc.gpsimd.collective_compute(
            "AllGather", OP.bypass,
            replica_groups=[list(range(N_CORES))],
            ins=[contribB.opt()], outs=[gatherB.opt()])

        # ================= Stage 2c: u (fills the gather shadow) ==========
        for ci in range(4):
            pu4 = ps.tile([P, 4, R], F32, tag="mm4", name=f"pu{ci}")
            for ui in range(4):
                ut = ci * 4 + ui
                for a in range(A):
                    nc.tensor.matmul(
                        pu4[:, ui, :],
                        wu_sb[:, 2 * a:2 * a + 2, ut * P:(ut + 1) * P],
                        hT[:, 2 * a:2 * a + 2, :],
                        perf_mode=DR, start=(a == 0), stop=(a == A - 1))
            for ui in range(4):
                ut = ci * 4 + ui
                nc.scalar.activation(out=uT[:, ut, :], in_=pu4[:, ui, :],
                                     func=AF.Silu,
                                     bias=uvb_sb[:, ut:ut + 1],
                                     scale=2.0 ** -6)

        # ================= Stage 3: scores + relu^2 =======================
        # gatherK block for core c: 32 rows of f8 k (4 p-rows per row)
        nc.sync.dma_start(
            kT_full,
            gatherK[:].rearrange("(c b) (four w) -> (b four) c w",
                                 b=CBK, four=4)[:P])
        for kq in range(KT // 4):
            pg = ps.tile([P, 4, R], F32, tag="mm4", name=f"pg{kq}")
            for j in range(4):
                kt = 4 * kq + j
                c, rb = kt // RT, kt % RT
                nc.tensor.matmul(pg[:, j, :],
                                 kT_full[:, c, rb * P:(rb + 1) * P],
                                 qT[:], start=True, stop=True)
            t_relu = tmp.tile([P, 4, R], BF16, tag="relu", bufs=2)
            nc.vector.tensor_scalar_max(t_relu, pg, 0.0)
            nc.vector.tensor_tensor(g_sb[:, 4 * kq:4 * kq + 4, :],
                                    t_relu, t_relu, OP.mult)

        # ================= Stage 4: attn = g @ v; y = u * attn ===========
        # fp8 DoubleRow over PAIRS of adjacent key tiles.  A-pairs
        # (4c, 4c+1) live in gatherA block c, B-pairs (4c+2, 4c+3) in
        # gatherB block c.  Four passes (A ch0, A ch1, B ch0, B ch1) with
        # SBUF accumulation of y, so only the two short B passes depend on
        # the late gatherB.
        def stripe(gsrc, c, ch):
            st = vstr.tile([P, 2, 1024], F8, tag="vstripe")
            nc.gpsimd.dma_start(
                st, gsrc[c * 2 * P:(c + 1) * 2 * P,
                         ch * 1024:(ch + 1) * 1024]
                .rearrange("(a p) e -> p a e", a=2))
            return st

        EC = 8

        def attn_pass(gsrc, koff, ch, accumulate):
            pa_lo = ps.tile([P, 4, R], F32, tag="mm4")
            pa_hi = ps.tile([P, 4, R], F32, tag="mm4")
            for c in range(N_CORES):
                st = stripe(gsrc, c, ch)
                gpair = g_sb[:, 4 * c + koff:4 * c + koff + 2, :]
                for ei in range(EC):
                    pa = pa_lo if ei < 4 else pa_hi
                    nc.tensor.matmul(pa[:, ei % 4, :],
                                     st[:, :, ei * P:(ei + 1) * P],
                                     gpair,
                                     perf_mode=DR,
                                     start=(c == 0), stop=(c == N_CORES - 1))
            for half, pa in enumerate((pa_lo, pa_hi)):
                usl = slice(ch * EC + half * 4, ch * EC + half * 4 + 4)
                if not accumulate:
                    nc.vector.scalar_tensor_tensor(
                        out=y_sb[:, usl, :], in0=pa, scalar=2.0 ** -10,
                        in1=uT[:, usl, :], op0=OP.mult, op1=OP.mult)
                else:
                    yb = tmp.tile([P, 4, R], BF16, tag="yb", bufs=2)
                    nc.vector.scalar_tensor_tensor(
                        out=yb, in0=pa, scalar=2.0 ** -10,
                        in1=uT[:, usl, :], op0=OP.mult, op1=OP.mult)
                    nc.vector.tensor_tensor(
                        y_sb[:, usl, :], y_sb[:, usl, :], yb, OP.add)

        for ch in range(2):
            attn_pass(gatherA, 0, ch, accumulate=False)
        for ch in range(2):
            attn_pass(gatherB, 2, ch, accumulate=True)

        # ================= Stage 5: out = y @ o_w.T + o_b + x ============
        outr3 = outr[:].rearrange("(t p) c -> p t c", p=P)
        for hc in range(2):
            po4 = ps.tile([P, 4, 512], F32, tag="mm4", name=f"po{hc}")
            for rt in range(RT):
                for t in range(UT // 2):
                    nc.tensor.matmul(
                        po4[:, rt, :],
                        y_sb[:, 2 * t:2 * t + 2, rt * P:(rt + 1) * P],
                        wo_sb[:, hc * UT + 2 * t:hc * UT + 2 * t + 2, :],
                        perf_mode=DR, start=(t == 0), stop=(t == UT // 2 - 1))
            for hh in range(2):
                o_sb = tmp.tile([P, 2, 512], F32, tag="osb")
                nc.vector.scalar_tensor_tensor(
                    out=o_sb, in0=po4[:, 2 * hh:2 * hh + 2, :],
                    scalar=2.0 ** -24,
                    in1=xpb_sb[:, 2 * hh:2 * hh + 2,
                               hc * 512:(hc + 1) * 512],
                    op0=OP.mult, op1=OP.add)
                nc.sync.dma_start(
                    outr3[:, 2 * hh:2 * hh + 2, hc * 512:(hc + 1) * 512],
                    o_sb)

    nc.finalize()
    return nc


_NC_CACHE = {}


def _get_nc(vb_zero):
    if vb_zero not in _NC_CACHE:
        _NC_CACHE[vb_zero] = build(vb_zero)
    return _NC_CACHE[vb_zero]


def _make_in_maps(inputs):
    import ml_dtypes
    f8 = ml_dtypes.float8_e4m3fn
    x = np.asarray(inputs["x"], dtype=np.float32)
    uv_w = np.asarray(inputs["uv_w"], dtype=np.float32)
    o_w = np.asarray(inputs["o_w"], dtype=np.float32)
    qk_w = np.asarray(inputs["qk_weight"], dtype=np.float32)
    qk_b = np.asarray(inputs["qk_bias"], dtype=np.float32)
    gamma = np.asarray(inputs["ln_gamma"], dtype=np.float32)
    beta = np.asarray(inputs["ln_beta"], dtype=np.float32)
    uv_b = np.asarray(inputs["uv_b"], dtype=np.float32)
    o_b = np.asarray(inputs["o_b"], dtype=np.float32)
    sq = np.float32(1.0 / np.sqrt(np.float32(128.0)))

    # fold gamma/beta into the uv projection:
    #   (z*gamma + beta) @ W.T = z @ (W*gamma).T + W@beta
    uv_w_f = uv_w * gamma[None, :]
    uv_b_f = (uv_b.astype(np.float64)
              + uv_w.astype(np.float64) @ beta.astype(np.float64)
              ).astype(np.float32)

    def to_pht(w, cols):
        # [cols, H] weight rows -> [P, HT, cols] with h = ht*128 + p
        return np.ascontiguousarray(
            w.T.reshape(HT, P, cols).transpose(1, 0, 2))

    wb8 = (to_pht(uv_w_f[2 * E:], S) * 64.0).astype(f8)
    wuv8 = np.stack([
        (to_pht(uv_w_f[E:2 * E], E) * 64.0).astype(f8),
        (to_pht(uv_w_f[:E], E) * 64.0).astype(f8)])
    # o_w [H, E] -> [P, 2*UT, 512] with (hc, et) interleaved: index
    # hc*UT + et, e = et*128 + p, columns = hc*512 + c
    wo = o_w.T.reshape(UT, P, 2, 512).transpose(1, 2, 0, 3).reshape(
        P, 2 * UT, 512)
    wo8 = np.ascontiguousarray(wo * 64.0).astype(f8)
    uvb17 = np.concatenate(
        [uv_b_f[:E].reshape(UT, P).T, uv_b_f[2 * E:].reshape(1, P).T],
        axis=1).astype(np.float32)
    uvb17 = np.ascontiguousarray(uvb17)

    shared = dict(
        wb8=wb8, wuv8=wuv8, wo8=wo8, uvb17=uvb17,
        vb=np.ascontiguousarray(uv_b_f[E:2 * E]),
        qs=np.ascontiguousarray(qk_w[0] * sq * 128.0),
        qb=np.ascontiguousarray(qk_b[0] * sq * 128.0),
        ks=np.ascontiguousarray(qk_w[1] * 128.0),
        kb=np.ascontiguousarray(qk_b[1] * 128.0),
        cwb=np.ascontiguousarray(uv_w_f[2 * E:].sum(axis=1)
                                 .astype(np.float32)),
    )
    xpb = x + o_b[None, :]
    maps = []
    for c in range(N_CORES):
        xc = x[c * R:(c + 1) * R]
        xt8 = np.ascontiguousarray(
            xc.T.reshape(HT, P, R).transpose(1, 0, 2)).astype(f8)
        maps.append(dict(shared, xt8=xt8,
                         xpb=np.ascontiguousarray(xpb[c * R:(c + 1) * R])))
    return maps


def run(inputs, trace=False, **kw):
    in_maps = _make_in_maps(inputs)
    nc = _get_nc(bool(np.all(in_maps[0]["vb"] == 0.0)))
    res = run_bass_kernel_spmd(nc, in_maps, list(range(N_CORES)),
                               trace=trace, **kw)
    out = np.concatenate([res.results[c]["out"] for c in range(N_CORES)],
                         axis=0)
    return out, res


def kernel(**inputs) -> np.ndarray:
    out, _ = run(inputs)
    return out
